# revision 10
# baseline (speedup 1.0000x reference)
"""Trainium2 Bass kernel for BotanHadamardTransform: y = x @ H, with
x [4, 4096, 4096] f32 and H [4096, 4096] f32 the normalized Sylvester
Hadamard matrix H_4096 / 64.

Algorithm (bf16 end-to-end, rel err ~4e-3 vs the 2e-2 gate):
H_4096 = H_8 (x) H_512 (Sylvester Kronecker nesting).  For a row v
viewed as [a=8, b=512]:
  1. FWHT over the a axis: 3 butterfly stages, done input-side
     (decimation-in-time) as 10 DVE tensor_tensor ops per r-tile on
     bf16 SBUF tiles (2x perf mode).  Stages 2/3 are split by
     chunk-halves so the first half of g3 (chunks 0..15 = a' 0..3)
     completes ~2/3 into the tile, letting the PE start early.
  2. per-a matmul with Hf = H[:512,:512] (= H_512/64, exact in bf16) on
     the PE, f32 PSUM accumulation over 4 k-chunks.  Each matmul writes
     a full psum bank (start=True clears has_written bank-wide); the
     bank packs n a'-slices x Rt columns, rhs gathered via a stride-4
     chunk slice.  ~44 dependency-free warm-up matmuls on scratch data
     run during the pipeline fill so the PE HAM clock gate stays at
     8/8 (idle >3.4us re-throttles to half clock).
  3. ScalarE evicts PSUM f32 -> bf16 SBUF; contiguous per-tile DMA out.

Data-parallel over 8 cores: core c owns 2048 rows.  Host packs per-core
pre-tiled bf16 blocks (flat dram, one contiguous block per r-tile):
  x block t: [128, 32, Rt] with x[t][p, c, i] = x_rows.T[c*128+p, r0+i]
  y block t: [128, 8, 4, Rt] with y[row=r0+i, col=a'*512+q*128+p]
           = y[t][p, a', q, i]
R-tile sizes [32, 64, 128, 256*7, 32]: small edge tiles shrink pipeline
fill (PE start) and drain (last DMA-out).
"""
import sys

sys.path.insert(0, "/opt/trn_rl_repo")

import numpy as np
import ml_dtypes

import concourse.bass as bass  # noqa: F401
import concourse.tile as tile
from concourse import bacc, mybir
from concourse.bass_utils import run_bass_kernel_spmd

BF16 = ml_dtypes.bfloat16

N_CORES = 8
N = 4096             # hidden dim
ROWS = 4 * 4096      # total rows
RC = ROWS // N_CORES  # rows per core = 2048

B = 512              # PE-contracted Kronecker factor (Hf = H_512/64)
A = N // B           # butterfly factor (8)
NCH = N // 128       # 32 k-chunks
SUB = B // 128       # accumulating matmuls per output chunk (4)

RTS = (32, 64, 128, 256, 256, 256, 256, 256, 256, 256, 32)
assert sum(RTS) == RC
N_WARM = 44          # HAM warm-up matmuls (N=512 each, ~216 ns apiece)


def _build():
    nc = bacc.Bacc("TRN2", target_bir_lowering=False, debug=False,
                   num_devices=N_CORES)
    bf = mybir.dt.bfloat16
    f32 = mybir.dt.float32

    x_ap = nc.dram_tensor("xt", [N * RC], bf, kind="ExternalInput").ap()
    hf_ap = nc.dram_tensor("hf", [128, SUB * B], bf,
                           kind="ExternalInput").ap()
    y_ap = nc.dram_tensor("yt", [N * RC], bf, kind="ExternalOutput").ap()

    with tile.TileContext(nc) as tc:
        with (
            tc.tile_pool(name="hfp", bufs=1) as hfp,
            tc.tile_pool(name="wrm", bufs=1) as wrmp,
            tc.tile_pool(name="xb", bufs=3) as xbp,
            tc.tile_pool(name="g1", bufs=2) as g1p,
            tc.tile_pool(name="g2", bufs=2) as g2p,
            tc.tile_pool(name="g3", bufs=2) as g3p,
            tc.tile_pool(name="yb", bufs=2) as ybp,
            tc.tile_pool(name="ps", bufs=3, space="PSUM") as psp,
            tc.tile_pool(name="pw", bufs=1, space="PSUM") as pwp,
        ):
            # HAM warm-up: dependency-free matmuls on zeroed scratch keep
            # the PE at full clock while the first tiles load/butterfly.
            wl = wrmp.tile([128, 128], bf, tag="wl")
            wr = wrmp.tile([128, 512], bf, tag="wr")
            pw = pwp.tile([128, 512], f32, tag="pw")
            nc.gpsimd.memset(wl[:], 0.0)
            nc.gpsimd.memset(wr[:], 0.0)
            for w in range(N_WARM):
                nc.tensor.matmul(pw[:], wl[:], wr[:], start=True, stop=True)

            # hf rides the scalar HWDGE queue so the sync queue starts the
            # first x-tile load immediately
            hf = hfp.tile([128, SUB * B], bf, tag="hf")
            nc.scalar.dma_start(hf[:], hf_ap)

            def hfblk(s, q):
                o = s * B + q * 128
                return hf[:, o:o + 128]

            off = 0
            for t, rt in enumerate(RTS):
                fd = NCH * rt
                xb = xbp.tile([128, fd], bf, tag="xb", name=f"xb{t}")
                nc.sync.dma_start(
                    xb[:], x_ap[off * 128:(off + fd) * 128].rearrange(
                        "(p f) -> p f", p=128))
                g1 = g1p.tile([128, fd], bf, tag="g1", name=f"g1{t}")
                g2 = g2p.tile([128, fd], bf, tag="g2", name=f"g2{t}")
                g3 = g3p.tile([128, fd], bf, tag="g3", name=f"g3{t}")

                def vi(tt, i):
                    return tt.rearrange("p (i c r) -> p i c r",
                                        i=i, c=NCH // i, r=rt)

                # stage 1: a-stride 4 -> pair 16-chunk halves
                xv, g1v = vi(xb, 1), vi(g1, 1)
                nc.vector.tensor_add(g1v[:, :, 0:16, :], xv[:, :, 0:16, :],
                                     xv[:, :, 16:32, :])
                nc.vector.tensor_sub(g1v[:, :, 16:32, :], xv[:, :, 0:16, :],
                                     xv[:, :, 16:32, :])
                # stages 2+3 run half-by-half (h = chunks 16h..16h+15) so
                # the PE can start on a' 0..3 while h=1 is still in flight
                g1v, g2v = vi(g1, 2), vi(g2, 2)
                g2v4, g3v4 = vi(g2, 4), vi(g3, 4)
                for h in range(2):
                    # stage 2: a-stride 2 within half h
                    nc.vector.tensor_add(g2v[:, h:h + 1, 0:8, :],
                                         g1v[:, h:h + 1, 0:8, :],
                                         g1v[:, h:h + 1, 8:16, :])
                    nc.vector.tensor_sub(g2v[:, h:h + 1, 8:16, :],
                                         g1v[:, h:h + 1, 0:8, :],
                                         g1v[:, h:h + 1, 8:16, :])
                    # stage 3: a-stride 1 within half h
                    i0, i1 = 2 * h, 2 * h + 2
                    nc.vector.tensor_add(g3v4[:, i0:i1, 0:4, :],
                                         g2v4[:, i0:i1, 0:4, :],
                                         g2v4[:, i0:i1, 4:8, :])
                    nc.vector.tensor_sub(g3v4[:, i0:i1, 4:8, :],
                                         g2v4[:, i0:i1, 0:4, :],
                                         g2v4[:, i0:i1, 4:8, :])

                g3c = g3.rearrange("p (c r) -> p c r", c=NCH, r=rt)
                bank_sl = 512 // rt            # a'-slices per psum bank
                tile_sl = min(A, 2 * bank_sl)  # a'-slices per psum tile
                ngrp = A // tile_sl            # psum tiles per q
                nmm = max(1, tile_sl // bank_sl)  # bank matmuls per s
                sl_mm = tile_sl // nmm         # a'-slices per matmul
                yb = ybp.tile([128, A * SUB * rt], bf, tag="yb",
                              name=f"yb{t}")
                ybv = yb.rearrange("p (a q r) -> p a q r", a=A, q=SUB, r=rt)
                for g in range(ngrp):
                    for q in range(SUB):
                        ps = psp.tile([128, 1024], f32, tag="ps",
                                      name=f"ps{t}_{g}_{q}")
                        pv = ps[:, 0:tile_sl * rt].rearrange(
                            "p (a r) -> p a r", a=tile_sl, r=rt)
                        for u in range(nmm):
                            for s in range(SUB):
                                a0 = g * tile_sl + u * sl_mm
                                c0 = a0 * SUB + s
                                nc.tensor.matmul(
                                    pv[:, u * sl_mm:(u + 1) * sl_mm, :],
                                    hfblk(s, q),
                                    g3c[:, c0:c0 + (sl_mm - 1) * SUB + 1:SUB, :],
                                    start=(s == 0), stop=(s == SUB - 1))
                        nc.scalar.copy(
                            ybv[:, g * tile_sl:(g + 1) * tile_sl, q, :],
                            pv)
                nc.scalar.dma_start(
                    y_ap[off * 128:(off + SUB * A * rt) * 128].rearrange(
                        "(p f) -> p f", p=128), yb[:])
                off += fd

    nc.compile()
    return nc


_prog = None


def _get_prog():
    global _prog
    if _prog is None:
        _prog = _build()
    return _prog


def prepare_in_maps(x, H):
    """Host-side pack: per-core pre-tiled bf16 blocks."""
    x = np.asarray(x, dtype=np.float32).reshape(ROWS, N)
    Hf = np.asarray(H, dtype=np.float32)[:B, :B].astype(BF16)
    hf_dev = np.ascontiguousarray(
        Hf.reshape(SUB, 128, B).transpose(1, 0, 2).reshape(128, SUB * B))
    in_maps = []
    for c in range(N_CORES):
        xc = x[c * RC:(c + 1) * RC].astype(BF16)        # [RC, N]
        xt = xc.T                                        # [N, RC] k-major
        blocks = []
        r0 = 0
        for rt in RTS:
            blk = xt[:, r0:r0 + rt].reshape(NCH, 128, rt).transpose(1, 0, 2)
            blocks.append(blk.reshape(-1))               # [128, NCH, rt]
            r0 += rt
        in_maps.append({"xt": np.concatenate(blocks), "hf": hf_dev})
    return in_maps


def _run(in_maps, trace=False):
    nc = _get_prog()
    return run_bass_kernel_spmd(nc, in_maps, core_ids=list(range(N_CORES)),
                                trace=trace)


def kernel(x, H):
    res = _run(prepare_in_maps(x, H))
    y = np.empty((ROWS, N), dtype=np.float32)
    for c in range(N_CORES):
        yt = res.results[c]["yt"]
        r0 = 0
        off = 0
        for rt in RTS:
            blk = yt[off * 128:(off + A * SUB * rt) * 128]
            blk = blk.reshape(128, A, SUB, rt).transpose(3, 1, 2, 0)
            y[c * RC + r0:c * RC + r0 + rt] = \
                blk.reshape(rt, N).astype(np.float32)
            r0 += rt
            off += A * SUB * rt
        assert off == N * RC // 128
    return y.reshape(4, 4096, N)


# revision 11
# speedup vs baseline: 1.0629x; 1.0629x over previous
"""Trainium2 Bass kernel for BotanHadamardTransform: y = x @ H, with
x [4, 4096, 4096] f32 and H [4096, 4096] f32 the normalized Sylvester
Hadamard matrix H_4096 / 64.

Algorithm (bf16 end-to-end, rel err ~4e-3 vs the 2e-2 gate):
H_4096 = H_8 (x) H_512 (Sylvester Kronecker nesting).  For a row v
viewed as [a=8, b=512]:
  1. FWHT over the a axis: 3 butterfly stages, done input-side
     (decimation-in-time) as 10 DVE tensor_tensor ops per r-tile on
     bf16 SBUF tiles (2x perf mode).  Stages 2/3 are split by
     chunk-halves so the first half of g3 (chunks 0..15 = a' 0..3)
     completes ~2/3 into the tile, letting the PE start early.
  2. per-a matmul with Hf = H[:512,:512] (= H_512/64, exact in bf16) on
     the PE, f32 PSUM accumulation over 4 k-chunks.  Each matmul writes
     a full psum bank (start=True clears has_written bank-wide); the
     bank packs n a'-slices x Rt columns, rhs gathered via a stride-4
     chunk slice.  ~44 dependency-free warm-up matmuls on scratch data
     run during the pipeline fill so the PE HAM clock gate stays at
     8/8 (idle >3.4us re-throttles to half clock).
  3. ScalarE evicts PSUM f32 -> bf16 SBUF; contiguous per-tile DMA out.

Data-parallel over 8 cores: core c owns 2048 rows.  Host packs per-core
pre-tiled bf16 blocks (flat dram, one contiguous block per r-tile):
  x block t: [128, 32, Rt] with x[t][p, c, i] = x_rows.T[c*128+p, r0+i]
  y block t: [128, 8, 4, Rt] with y[row=r0+i, col=a'*512+q*128+p]
           = y[t][p, a', q, i]
R-tile sizes [32, 64, 128, 256*7, 32]: small edge tiles shrink pipeline
fill (PE start) and drain (last DMA-out).
"""
import sys

sys.path.insert(0, "/opt/trn_rl_repo")

import numpy as np
import ml_dtypes

import concourse.bass as bass  # noqa: F401
import concourse.tile as tile
from concourse import bacc, mybir
from concourse.bass_utils import run_bass_kernel_spmd

BF16 = ml_dtypes.bfloat16

N_CORES = 8
N = 4096             # hidden dim
ROWS = 4 * 4096      # total rows
RC = ROWS // N_CORES  # rows per core = 2048

B = 512              # PE-contracted Kronecker factor (Hf = H_512/64)
A = N // B           # butterfly factor (8)
NCH = N // 128       # 32 k-chunks
SUB = B // 128       # accumulating matmuls per output chunk (4)

RTS = (32, 32, 64) + (128,) * 15
assert sum(RTS) == RC
N_WARM = 28          # HAM warm-up matmuls (N=512 each, ~216 ns apiece)


def _build():
    nc = bacc.Bacc("TRN2", target_bir_lowering=False, debug=False,
                   num_devices=N_CORES)
    bf = mybir.dt.bfloat16
    f32 = mybir.dt.float32

    x_ap = nc.dram_tensor("xt", [N * RC], bf, kind="ExternalInput").ap()
    hf_ap = nc.dram_tensor("hf", [128, SUB * B], bf,
                           kind="ExternalInput").ap()
    y_ap = nc.dram_tensor("yt", [N * RC], bf, kind="ExternalOutput").ap()

    with tile.TileContext(nc) as tc:
        with (
            tc.tile_pool(name="hfp", bufs=1) as hfp,
            tc.tile_pool(name="wrm", bufs=1) as wrmp,
            tc.tile_pool(name="xb", bufs=4) as xbp,
            tc.tile_pool(name="g1", bufs=2) as g1p,
            tc.tile_pool(name="g2", bufs=2) as g2p,
            tc.tile_pool(name="g3", bufs=3) as g3p,
            tc.tile_pool(name="yb", bufs=2) as ybp,
            tc.tile_pool(name="ps", bufs=3, space="PSUM") as psp,
            tc.tile_pool(name="pw", bufs=1, space="PSUM") as pwp,
        ):
            # HAM warm-up: dependency-free matmuls on zeroed scratch keep
            # the PE at full clock while the first tiles load/butterfly.
            wl = wrmp.tile([128, 128], bf, tag="wl")
            wr = wrmp.tile([128, 512], bf, tag="wr")
            pw = pwp.tile([128, 512], f32, tag="pw")
            nc.gpsimd.memset(wl[:], 0.0)
            nc.gpsimd.memset(wr[:], 0.0)
            for w in range(N_WARM):
                nc.tensor.matmul(pw[:], wl[:], wr[:], start=True, stop=True)

            # hf rides the scalar HWDGE queue so the sync queue starts the
            # first x-tile load immediately
            hf = hfp.tile([128, SUB * B], bf, tag="hf")
            nc.scalar.dma_start(hf[:], hf_ap)

            def hfblk(s, q):
                o = s * B + q * 128
                return hf[:, o:o + 128]

            off = 0
            for t, rt in enumerate(RTS):
                fd = NCH * rt
                xb = xbp.tile([128, fd], bf, tag="xb", name=f"xb{t}")
                nc.sync.dma_start(
                    xb[:], x_ap[off * 128:(off + fd) * 128].rearrange(
                        "(p f) -> p f", p=128))
                g1 = g1p.tile([128, fd], bf, tag="g1", name=f"g1{t}")
                g2 = g2p.tile([128, fd], bf, tag="g2", name=f"g2{t}")
                g3 = g3p.tile([128, fd], bf, tag="g3", name=f"g3{t}")

                def vi(tt, i):
                    return tt.rearrange("p (i c r) -> p i c r",
                                        i=i, c=NCH // i, r=rt)

                # stage 1: a-stride 4 -> pair 16-chunk halves
                xv, g1v = vi(xb, 1), vi(g1, 1)
                nc.vector.tensor_add(g1v[:, :, 0:16, :], xv[:, :, 0:16, :],
                                     xv[:, :, 16:32, :])
                nc.vector.tensor_sub(g1v[:, :, 16:32, :], xv[:, :, 0:16, :],
                                     xv[:, :, 16:32, :])
                # stages 2+3 run half-by-half (h = chunks 16h..16h+15) so
                # the PE can start on a' 0..3 while h=1 is still in flight
                g1v, g2v = vi(g1, 2), vi(g2, 2)
                g2v4, g3v4 = vi(g2, 4), vi(g3, 4)
                for h in range(2):
                    # stage 2: a-stride 2 within half h
                    nc.vector.tensor_add(g2v[:, h:h + 1, 0:8, :],
                                         g1v[:, h:h + 1, 0:8, :],
                                         g1v[:, h:h + 1, 8:16, :])
                    nc.vector.tensor_sub(g2v[:, h:h + 1, 8:16, :],
                                         g1v[:, h:h + 1, 0:8, :],
                                         g1v[:, h:h + 1, 8:16, :])
                    # stage 3: a-stride 1 within half h
                    i0, i1 = 2 * h, 2 * h + 2
                    nc.vector.tensor_add(g3v4[:, i0:i1, 0:4, :],
                                         g2v4[:, i0:i1, 0:4, :],
                                         g2v4[:, i0:i1, 4:8, :])
                    nc.vector.tensor_sub(g3v4[:, i0:i1, 4:8, :],
                                         g2v4[:, i0:i1, 0:4, :],
                                         g2v4[:, i0:i1, 4:8, :])

                g3c = g3.rearrange("p (c r) -> p c r", c=NCH, r=rt)
                bank_sl = 512 // rt            # a'-slices per psum bank
                tile_sl = min(A, 2 * bank_sl)  # a'-slices per psum tile
                ngrp = A // tile_sl            # psum tiles per q
                nmm = max(1, tile_sl // bank_sl)  # bank matmuls per s
                sl_mm = tile_sl // nmm         # a'-slices per matmul
                yb = ybp.tile([128, A * SUB * rt], bf, tag="yb",
                              name=f"yb{t}")
                ybv = yb.rearrange("p (a q r) -> p a q r", a=A, q=SUB, r=rt)
                for g in range(ngrp):
                    for q in range(SUB):
                        ps = psp.tile([128, 1024], f32, tag="ps",
                                      name=f"ps{t}_{g}_{q}")
                        pv = ps[:, 0:tile_sl * rt].rearrange(
                            "p (a r) -> p a r", a=tile_sl, r=rt)
                        for u in range(nmm):
                            for s in range(SUB):
                                a0 = g * tile_sl + u * sl_mm
                                c0 = a0 * SUB + s
                                nc.tensor.matmul(
                                    pv[:, u * sl_mm:(u + 1) * sl_mm, :],
                                    hfblk(s, q),
                                    g3c[:, c0:c0 + (sl_mm - 1) * SUB + 1:SUB, :],
                                    start=(s == 0), stop=(s == SUB - 1))
                        nc.scalar.copy(
                            ybv[:, g * tile_sl:(g + 1) * tile_sl, q, :],
                            pv)
                nc.scalar.dma_start(
                    y_ap[off * 128:(off + SUB * A * rt) * 128].rearrange(
                        "(p f) -> p f", p=128), yb[:])
                off += fd

    nc.compile()
    return nc


_prog = None


def _get_prog():
    global _prog
    if _prog is None:
        _prog = _build()
    return _prog


def prepare_in_maps(x, H):
    """Host-side pack: per-core pre-tiled bf16 blocks."""
    x = np.asarray(x, dtype=np.float32).reshape(ROWS, N)
    Hf = np.asarray(H, dtype=np.float32)[:B, :B].astype(BF16)
    hf_dev = np.ascontiguousarray(
        Hf.reshape(SUB, 128, B).transpose(1, 0, 2).reshape(128, SUB * B))
    in_maps = []
    for c in range(N_CORES):
        xc = x[c * RC:(c + 1) * RC].astype(BF16)        # [RC, N]
        xt = xc.T                                        # [N, RC] k-major
        blocks = []
        r0 = 0
        for rt in RTS:
            blk = xt[:, r0:r0 + rt].reshape(NCH, 128, rt).transpose(1, 0, 2)
            blocks.append(blk.reshape(-1))               # [128, NCH, rt]
            r0 += rt
        in_maps.append({"xt": np.concatenate(blocks), "hf": hf_dev})
    return in_maps


def _run(in_maps, trace=False):
    nc = _get_prog()
    return run_bass_kernel_spmd(nc, in_maps, core_ids=list(range(N_CORES)),
                                trace=trace)


def kernel(x, H):
    res = _run(prepare_in_maps(x, H))
    y = np.empty((ROWS, N), dtype=np.float32)
    for c in range(N_CORES):
        yt = res.results[c]["yt"]
        r0 = 0
        off = 0
        for rt in RTS:
            blk = yt[off * 128:(off + A * SUB * rt) * 128]
            blk = blk.reshape(128, A, SUB, rt).transpose(3, 1, 2, 0)
            y[c * RC + r0:c * RC + r0 + rt] = \
                blk.reshape(rt, N).astype(np.float32)
            r0 += rt
            off += A * SUB * rt
        assert off == N * RC // 128
    return y.reshape(4, 4096, N)
